# revision 34
# baseline (speedup 1.0000x reference)
"""AttnBlock (GroupNorm + single-head 1x1-conv attention + residual) on 8 TRN2 cores.

Data-parallel over batch: core i processes x[i] (512, 64*64) entirely on-chip.

Math (per batch item, N = 64*64 = 4096 spatial positions, C = 512 channels).
GroupNorm is folded into the weights instead of materializing the normalized
activations: with R = a*X + b (a, b the per-channel GN affine computed from
batch statistics on device),

  scores[n,m] = R_n^T Ws R_m            (Ws = Wq^T Wk, host-precomputed)
              = X_n^T (a Ws a) X_m  + [per-n terms that cancel in softmax]
                + b^T Ws a X_m  (~1e-5 relative, dropped)
  V           = Wk R = (Wk a) X + Wk b;  the Wk b part is constant over m and
                softmax rows sum to 1, so it contributes exactly
                proj_w @ Wk b to the output -> folded into the proj bias via
                the host-precomputed (proj_w @ k_w)^T and the device-computed b.

So the device only ever needs x8 = fp8(X) (cast on DMA arrival, no stats
dependency), plus cheap per-rep weight rescales W2 = a x Ws x a and
KA = a x Wk^T. All big matmuls run in fp8e4m3 with perf_mode=DoubleRow
(256-deep contraction); accumulation is fp32 in PSUM. The softmax denominator
d[n] is accumulated with an all-ones DoubleRow matmul in the transposed [m, n]
layout; 1/d (fast-approx reciprocal straight from PSUM) is applied after the
linear projection. The residual x is re-streamed from HBM per n-window rather
than held in SBUF, which frees enough SBUF to double-buffer every cross-rep
tensor: in the repeated (chained) execution the entire DMA + stats + rescale
head of rep i+1 overlaps rep i's attention matmuls.
"""
import sys

sys.path.insert(0, "/opt/trn_rl_repo")

import numpy as np
import ml_dtypes

import concourse.bass as bass
import concourse.bacc as bacc
import concourse.mybir as mybir
import concourse.tile as tile
from concourse import bass_utils

F32 = mybir.dt.float32
FP8 = mybir.dt.float8e4
DR = mybir.MatmulPerfMode.DoubleRow
AF = mybir.ActivationFunctionType
OP = mybir.AluOpType

B = 8
C = 512
N = 4096          # 64*64 spatial
GROUPS = 32
GSIZE = 16        # channels per group
EPS = 1e-6
CCH = 4           # channel chunks of 128
NCH = 8           # n chunks of 512
MT = 32           # m tiles of 128
P = 128
NW = 512          # matmul free dim / n-chunk width
XCH = 2048        # x arrival chunk width (2 chunks per cp)
INV_SQRT_C = 1.0 / float(np.sqrt(C))

_BUILD_CACHE = {}


def _build(use_amt: bool, reps: int = 1):
    nc = bacc.Bacc("TRN2", target_bir_lowering=False)

    x_in = nc.dram_tensor("x_in", [C, N], F32, kind="ExternalInput")
    wst_d = nc.dram_tensor("wst", [C, C], FP8, kind="ExternalInput")
    kwt_d = nc.dram_tensor("kwt", [C, C], FP8, kind="ExternalInput")
    pwt_d = nc.dram_tensor("pwt", [C, C], FP8, kind="ExternalInput")
    pwkt_d = nc.dram_tensor("pwkt", [C, C], FP8, kind="ExternalInput")
    gamma_d = nc.dram_tensor("gamma_r", [P, CCH], F32, kind="ExternalInput")
    beta64_d = nc.dram_tensor("beta64_r", [P, CCH], F32, kind="ExternalInput")
    pb_d = nc.dram_tensor("pb_r", [P, CCH], F32, kind="ExternalInput")
    ones_d = nc.dram_tensor("ones_b", [P, 2 * P], FP8, kind="ExternalInput")
    g_d = nc.dram_tensor("g32", [P, CCH * GROUPS], F32, kind="ExternalInput")
    g2_d = nc.dram_tensor("g232", [GROUPS, CCH * P], F32, kind="ExternalInput")
    oneh_d = nc.dram_tensor("onehot", [GROUPS, NW], F32, kind="ExternalInput")
    ones32_d = nc.dram_tensor("ones32", [GROUPS, P], F32, kind="ExternalInput")
    if use_amt:
        qbb_d = nc.dram_tensor("qbb", [P, NW], F32, kind="ExternalInput")
    out_d = nc.dram_tensor("out", [C, N], F32, kind="ExternalOutput")

    with tile.TileContext(nc) as tc:
        # ---- persistent pools ----
        const = tc.alloc_tile_pool(name="const", bufs=1)
        xs_pool = tc.alloc_tile_pool(name="xs_pool", bufs=4)
        r8_pool = tc.alloc_tile_pool(name="r8_pool", bufs=2)
        u8_pool = tc.alloc_tile_pool(name="u8_pool", bufs=2)
        vt_pool = tc.alloc_tile_pool(name="vt_pool", bufs=MT // 2)
        et_pool = tc.alloc_tile_pool(name="et_pool", bufs=8)
        es_pool = tc.alloc_tile_pool(name="es_pool", bufs=4)
        osb_pool = tc.alloc_tile_pool(name="osb_pool", bufs=3)
        rd_pool = tc.alloc_tile_pool(name="rd_pool", bufs=2)
        outsb_pool = tc.alloc_tile_pool(name="outsb_pool", bufs=2)
        res_pool = tc.alloc_tile_pool(name="res_pool", bufs=2)
        stat_pool = tc.alloc_tile_pool(name="stat_pool", bufs=2)
        wsc_pool = tc.alloc_tile_pool(name="wsc_pool", bufs=2)
        psx_pool = tc.alloc_tile_pool(name="psx_pool", bufs=1, space="PSUM")

        wst_sb = const.tile([P, CCH, NW], FP8)
        kwt_sb = const.tile([P, CCH, NW], FP8)
        pwt_sb = const.tile([P, CCH, NW], FP8)
        pwkt_sb = const.tile([P, CCH, NW], FP8)
        gamma_sb = const.tile([P, CCH], F32)
        beta64_sb = const.tile([P, CCH], F32)
        pb_sb = const.tile([P, CCH], F32)
        ones_sb = const.tile([P, 2, P], FP8)
        g_sb = const.tile([P, CCH * GROUPS], F32)
        g2_sb = const.tile([GROUPS, CCH * P], F32)
        oneh_sb = const.tile([GROUPS, NW], F32)
        ones32_sb = const.tile([GROUPS, P], F32)
        nc.sync.dma_start(out=wst_sb,
                          in_=wst_d[:, :].rearrange('(c p) w -> p c w', p=P))
        nc.sync.dma_start(out=kwt_sb,
                          in_=kwt_d[:, :].rearrange('(c p) w -> p c w', p=P))
        nc.sync.dma_start(out=pwt_sb,
                          in_=pwt_d[:, :].rearrange('(c p) w -> p c w', p=P))
        nc.sync.dma_start(out=pwkt_sb,
                          in_=pwkt_d[:, :].rearrange('(c p) w -> p c w', p=P))
        nc.sync.dma_start(out=gamma_sb, in_=gamma_d[:, :])
        nc.sync.dma_start(out=beta64_sb, in_=beta64_d[:, :])
        nc.sync.dma_start(out=pb_sb, in_=pb_d[:, :])
        nc.sync.dma_start(out=ones_sb, in_=ones_d[:, :].rearrange('p (a b) -> p a b', a=2))
        nc.sync.dma_start(out=g_sb, in_=g_d[:, :])
        nc.sync.dma_start(out=g2_sb, in_=g2_d[:, :])
        nc.sync.dma_start(out=oneh_sb, in_=oneh_d[:, :])
        nc.sync.dma_start(out=ones32_sb, in_=ones32_d[:, :])
        if use_amt:
            qbb_sb = const.tile([P, NW], F32)
            nc.sync.dma_start(out=qbb_sb, in_=qbb_d[:, :])

        def head_state(rep):
            """Per-generation tiles for the stats/rescale chain of rep `rep`."""
            return {
                "r8": r8_pool.tile([P, CCH, N], FP8, tag="r8", name="r8"),
                "A_col": stat_pool.tile([P, CCH], F32, tag="A_col", name="A_col"),
                "B8": stat_pool.tile([P, CCH, 1], FP8, tag="B8", name="B8"),
                "rhsb": stat_pool.tile([GROUPS, NW], F32, tag="rhsb", name="rhsb"),
                "s_all": stat_pool.tile([P, CCH, 2], F32, tag="s_all", name="s_all"),
                "bnst": stat_pool.tile([P, CCH, 8, 6], F32, tag="bnst", name="bnst"),
                "w2st": wsc_pool.tile([P, CCH, NW], FP8, tag="w2st", name="w2st"),
                "kwta": wsc_pool.tile([P, CCH, NW], FP8, tag="kwta", name="kwta"),
                "Ab": wsc_pool.tile([P, NW], F32, tag="Ab", name="Ab"),
                "pbe": wsc_pool.tile([P, CCH], F32, tag="pbe", name="pbe"),
            }

        def emit_head_piece(hs, piece, ppool):
            """piece 0..3: DMA + stats + cast for channel chunk cp=piece, plus
            that chunk's group-stat chain. piece 4: weight rescales."""
            if piece < CCH:
                cp = piece
                # two x chunks of [P, XCH] for this cp
                for half in range(2):
                    xs = xs_pool.tile([P, XCH], F32, tag="xs", name="xs")
                    nc.sync.dma_start(
                        out=xs,
                        in_=x_in[cp * P:(cp + 1) * P, half * XCH:(half + 1) * XCH])
                    for s in range(4):
                        nc.vector.bn_stats(out=hs["bnst"][:, cp, half * 4 + s, :],
                                           in_=xs[:, s * NW:(s + 1) * NW])
                    # cast to fp8 on arrival (no stats dependency). ACT is the
                    # only engine with a fast fp32->fp8 path (~1.1us per half
                    # chunk vs ~4us on DVE/GpSimd), and it has slack.
                    off = half * XCH
                    for j in range(2):
                        nc.scalar.copy(
                            out=hs["r8"][:, cp, off + j * 1024:off + (j + 1) * 1024],
                            in_=xs[:, j * 1024:(j + 1) * 1024])
                # per-channel [mean, E[x^2]] column for this cp
                with tc.tile_pool(name=f"st{cp}", bufs=1) as st:
                    mv = st.tile([P, 2], F32, tag="mv")
                    nc.vector.bn_aggr(out=mv, in_=hs["bnst"][:, cp, :, :])
                    nc.vector.tensor_copy(out=hs["s_all"][:, cp, 0:1], in_=mv[:, 0:1])
                    nc.vector.scalar_tensor_tensor(
                        out=hs["s_all"][:, cp, 1:2], in0=mv[:, 0:1], scalar=mv[:, 0:1],
                        in1=mv[:, 1:2], op0=OP.mult, op1=OP.add)
            else:
                # piece 4: group chain + weight rescales + proj-bias fold
                with tc.tile_pool(name="stg", bufs=1) as st:
                    ps32 = ppool.tile([GROUPS, 2], F32, tag="psx", name="ps32")
                    for cp in range(CCH):
                        nc.tensor.matmul(
                            ps32, lhsT=g_sb[:, cp * GROUPS:(cp + 1) * GROUPS],
                            rhs=hs["s_all"][:, cp, :],
                            start=(cp == 0), stop=(cp == CCH - 1))
                    mu = st.tile([GROUPS, 1], F32, tag="mu")
                    nc.vector.tensor_scalar_mul(out=mu, in0=ps32[:, 0:1], scalar1=1.0 / GSIZE)
                    ex2 = st.tile([GROUPS, 1], F32, tag="ex2")
                    nc.vector.tensor_scalar_mul(out=ex2, in0=ps32[:, 1:2], scalar1=1.0 / GSIZE)
                    musq = st.tile([GROUPS, 1], F32, tag="musq")
                    nc.vector.tensor_mul(out=musq, in0=mu, in1=mu)
                    veps = st.tile([GROUPS, 1], F32, tag="veps")
                    nc.vector.scalar_tensor_tensor(
                        out=veps, in0=ex2, scalar=EPS, in1=musq, op0=OP.add, op1=OP.subtract)
                    sd = st.tile([GROUPS, 1], F32, tag="sd")
                    nc.scalar.activation(out=sd, in_=veps, func=AF.Sqrt)
                    rs0 = st.tile([GROUPS, 1], F32, tag="rs0")
                    nc.vector.reciprocal(out=rs0, in_=sd)
                    # one Newton step: rs1 = rs0*(1.5 - 0.5*veps*rs0^2)
                    t1 = st.tile([GROUPS, 1], F32, tag="t1")
                    nc.vector.tensor_mul(out=t1, in0=rs0, in1=rs0)
                    t2 = st.tile([GROUPS, 1], F32, tag="t2")
                    nc.vector.tensor_mul(out=t2, in0=t1, in1=veps)
                    t3 = st.tile([GROUPS, 1], F32, tag="t3")
                    nc.vector.tensor_scalar(
                        out=t3, in0=t2, scalar1=-0.5, scalar2=1.5, op0=OP.mult, op1=OP.add)
                    rs1 = st.tile([GROUPS, 1], F32, tag="rs1")
                    nc.vector.tensor_mul(out=rs1, in0=t3, in1=rs0)
                    # group-level rsqrt rows for the free-side broadcast
                    nc.vector.tensor_scalar_mul(out=hs["rhsb"], in0=oneh_sb,
                                                scalar1=rs1)
                    w_sb = st.tile([GROUPS, 2], F32, tag="w_sb")
                    nc.vector.tensor_copy(out=w_sb[:, 0:1], in_=rs1)
                    nc.vector.tensor_copy(out=w_sb[:, 1:2], in_=mu)
                    pAb = ppool.tile([P, NW], F32, tag="psx", name="pAb")
                    nc.tensor.matmul(pAb, lhsT=ones32_sb, rhs=hs["rhsb"],
                                     start=True, stop=True)
                    nc.vector.tensor_copy(out=hs["Ab"], in_=pAb)
                    for cp in range(CCH):
                        psp2 = ppool.tile([P, 2], F32, tag="psx", name="psp2")
                        nc.tensor.matmul(psp2,
                                         lhsT=g2_sb[:, cp * P:(cp + 1) * P],
                                         rhs=w_sb, start=True, stop=True)
                        # a = gamma * rsqrt  (per-partition column of A_col)
                        nc.vector.tensor_mul(out=hs["A_col"][:, cp:cp + 1],
                                             in0=gamma_sb[:, cp:cp + 1],
                                             in1=psp2[:, 0:1])
                        tb = st.tile([P, 1], F32, tag="tb")
                        nc.vector.tensor_mul(out=tb, in0=psp2[:, 1:2],
                                             in1=hs["A_col"][:, cp:cp + 1])
                        # b8 = fp8(64 * (beta - mu*a)) = fp8(tb*-64 + beta64)
                        nc.vector.scalar_tensor_tensor(
                            out=hs["B8"][:, cp, :], in0=tb, scalar=-64.0,
                            in1=beta64_sb[:, cp:cp + 1], op0=OP.mult, op1=OP.add)
                    for cp in range(CCH):
                        nc.vector.scalar_tensor_tensor(
                            out=hs["w2st"][:, cp, :], in0=wst_sb[:, cp, :],
                            scalar=hs["A_col"][:, cp:cp + 1], in1=hs["Ab"],
                            op0=OP.mult, op1=OP.mult)
                        nc.scalar.activation(
                            out=hs["kwta"][:, cp, :], in_=kwt_sb[:, cp, :],
                            func=AF.Copy, scale=hs["A_col"][:, cp:cp + 1])
                    pbps = ppool.tile([P, CCH], F32, tag="psx", name="pbps")
                    for oc in range(CCH):
                        for ks in (0, 2):
                            nc.tensor.matmul(
                                pbps[:, oc:oc + 1],
                                lhsT=pwkt_sb[:, ks:ks + 2, oc * P:(oc + 1) * P],
                                rhs=hs["B8"][:, ks:ks + 2, :],
                                start=(ks == 0), stop=(ks == 2), perf_mode=DR)
                    # pb_eff = pb + (pwk @ b8)/64
                    nc.vector.scalar_tensor_tensor(
                        out=hs["pbe"], in0=pbps, scalar=1.0 / 64.0, in1=pb_sb,
                        op0=OP.mult, op1=OP.add)

        def emit_stage2(hs):
            """u8 = W2 x8, vt tiles = (x8^T KA); returns (vt_sb list, amt or None)."""
            r8_sb, w2st, kwta = hs["r8"], hs["w2st"], hs["kwta"]
            vt_sb = []
            amt_sb = None
            if use_amt:
                amt_sb = stat_pool.tile([P, MT], F32, tag="amt")
                ascr_sb = stat_pool.tile([P, NW], F32, tag="ascr")
            u8_sb = u8_pool.tile([P, CCH, N], FP8, tag="u8", name="u8")
            with tc.tile_pool(name="psv", bufs=7, space="PSUM") as psv_pool:
                for cq in range(CCH):
                    for mc in range(NCH):
                        psv = psv_pool.tile([P, NW], F32, tag="psv")
                        for ks in (0, 2):
                            nc.tensor.matmul(
                                psv,
                                lhsT=w2st[:, ks:ks + 2, cq * P:(cq + 1) * P],
                                rhs=r8_sb[:, ks:ks + 2, mc * NW:(mc + 1) * NW],
                                start=(ks == 0), stop=(ks == 2), perf_mode=DR)
                        # the first drains go to ACT: at the rep boundary the
                        # DVE queue still holds the previous rep's tail while
                        # ACT is idle (its exps finished during the flush).
                        k = cq * NCH + mc
                        if k < 22 or k % 2 == 1:
                            nc.scalar.copy(
                                out=u8_sb[:, cq, mc * NW:(mc + 1) * NW], in_=psv)
                        else:
                            nc.vector.tensor_copy(
                                out=u8_sb[:, cq, mc * NW:(mc + 1) * NW], in_=psv)
                for mt in range(MT):
                    if mt % 2 == 0:
                        vt_t = vt_pool.tile([P, 2, NW], FP8, tag="vt", name="vt")
                        vt_sb.append(vt_t)
                    psv = psv_pool.tile([P, NW], F32, tag="psv")
                    for ks in (0, 2):
                        nc.tensor.matmul(
                            psv,
                            lhsT=r8_sb[:, ks:ks + 2, mt * P:(mt + 1) * P],
                            rhs=kwta[:, ks:ks + 2, :],
                            start=(ks == 0), stop=(ks == 2), perf_mode=DR)
                    if mt % 4 == 3:
                        nc.scalar.copy(out=vt_sb[mt // 2][:, mt % 2, :], in_=psv)
                    else:
                        nc.vector.tensor_copy(out=vt_sb[mt // 2][:, mt % 2, :], in_=psv)
                    if use_amt:
                        nc.vector.scalar_tensor_tensor(
                            out=ascr_sb, in0=vt_sb[mt // 2][:, mt % 2, :],
                            scalar=INV_SQRT_C, in1=qbb_sb,
                            op0=OP.mult, op1=OP.mult, accum_out=amt_sb[:, mt:mt + 1])
            return u8_sb, vt_sb, amt_sb

        def emit_stage3(hs, u8_sb, vt_sb, amt_sb, hook, prev_state):
            r8_sb = hs["r8"]
            with tc.tile_pool(name="pss", bufs=3, space="PSUM") as pss_pool, \
                 tc.tile_pool(name="pso", bufs=1, space="PSUM") as pso_pool:

                def emit_proj(hs_cur, state, oc, outsb_t):
                    # osb3 already carries the 1/d normalization, so the proj
                    # output only needs bias + residual.
                    osb3, res_t, pnch, pbe_t = state
                    psp = psx_pool.tile([P, NW], F32, tag="psx", name="psp")
                    for ks in (0, 2):
                        nc.tensor.matmul(
                            psp,
                            lhsT=pwt_sb[:, ks:ks + 2, oc * P:(oc + 1) * P],
                            rhs=osb3[:, ks:ks + 2, :],
                            start=(ks == 0), stop=(ks == 2), perf_mode=DR)
                    nc.vector.scalar_tensor_tensor(
                        out=outsb_t[:, oc, :], in0=psp,
                        scalar=pbe_t[:, oc:oc + 1],
                        in1=res_t[:, oc, :], op0=OP.add, op1=OP.add)

                def finish_outsb(outsb_t, pnch):
                    nc.sync.dma_start(
                        out=out_d[:, pnch * NW:(pnch + 1) * NW]
                        .rearrange('(c p) w -> p c w', p=P),
                        in_=outsb_t)

                state = prev_state
                proj_slots = {1: 0, 3: 1, 5: 2, 7: 3}
                DEPTH = 4
                NPAIR = MT // 2
                for nch in range(NCH):
                    # residual stream for this nch (consumed by projs one nch later)
                    res_t = res_pool.tile([P, CCH, NW], F32, tag="res", name="res")
                    nc.sync.dma_start(
                        out=res_t,
                        in_=x_in[:, nch * NW:(nch + 1) * NW]
                        .rearrange('(c p) w -> p c w', p=P))
                    outsb_t = outsb_pool.tile([P, CCH, NW], F32, tag="outsb",
                                              name="outsb") if state is not None else None
                    pso_tiles = [pso_pool.tile([P, NW], F32, tag=f"pso{cs}",
                                               name=f"pso{cs}") for cs in range(CCH)]
                    psd_t = None
                    pend = []
                    es_tiles = []
                    es2_tiles = []
                    n_d = 0
                    cur_et = None
                    prev_et = None
                    for mt in range(MT):
                        pss = pss_pool.tile([P, NW], F32, tag="pss")
                        for ks in (0, 2):
                            nc.tensor.matmul(
                                pss,
                                lhsT=u8_sb[:, ks:ks + 2, mt * P:(mt + 1) * P],
                                rhs=r8_sb[:, ks:ks + 2, nch * NW:(nch + 1) * NW],
                                start=(ks == 0), stop=(ks == 2),
                                perf_mode=DR)
                        if mt % 2 == 0:
                            cur_et = et_pool.tile([P, 2, NW], FP8, tag="et", name="et")
                        if use_amt:
                            nc.scalar.activation(out=cur_et[:, mt % 2, :], in_=pss,
                                                 func=AF.Exp, scale=INV_SQRT_C,
                                                 bias=amt_sb[:, mt:mt + 1])
                        else:
                            nc.scalar.activation(out=cur_et[:, mt % 2, :], in_=pss,
                                                 func=AF.Exp, scale=INV_SQRT_C)
                        if state is not None and mt in proj_slots:
                            emit_proj(hs, state, proj_slots[mt], outsb_t)
                            if proj_slots[mt] == CCH - 1:
                                finish_outsb(outsb_t, state[2])

                        if mt % 2 == 1:
                            k = mt // 2
                            # tree-sum exp tiles on DVE so the denominator
                            # needs a quarter of the PE matmuls
                            if k % 2 == 1:
                                es_t = es_pool.tile([P, 2, NW], FP8, tag="es",
                                                      name="es")
                                nc.gpsimd.tensor_add(out=es_t, in0=prev_et,
                                                     in1=cur_et)
                                es_tiles.append(es_t)
                                if k % 4 == 3:
                                    es2_t = es_pool.tile([P, 2, NW], FP8,
                                                         tag="es2", name="es2")
                                    nc.gpsimd.tensor_add(out=es2_t,
                                                         in0=es_tiles[-2],
                                                         in1=es_tiles[-1])
                                    es2_tiles.append(es2_t)
                            prev_et = cur_et
                            pend.append((cur_et, k))
                            if len(pend) > DEPTH:
                                p_et, pt = pend.pop(0)
                                if pt % 4 == 3 and n_d < len(es2_tiles):
                                    if psd_t is None:
                                        psd_t = psx_pool.tile([P, NW], F32,
                                                              tag="psx", name="psd")
                                    nc.tensor.matmul(psd_t, lhsT=ones_sb,
                                                     rhs=es2_tiles[n_d],
                                                     start=(n_d == 0), stop=False,
                                                     perf_mode=DR)
                                    n_d += 1
                                for cs in range(CCH):
                                    nc.tensor.matmul(
                                        pso_tiles[cs],
                                        lhsT=vt_sb[pt][:, :, cs * P:(cs + 1) * P],
                                        rhs=p_et, start=(pt == 0), stop=False,
                                        perf_mode=DR)
                    # flush: remaining d-matmuls early (so 1/d is ready soon),
                    # but the last one (whose es2-add trails the final exp)
                    # goes after the first PV chunk to keep PE fed. PV matmuls
                    # are grouped by output chunk so each osb drain starts as
                    # soon as its chunk completes.
                    NES = NPAIR // 4
                    while n_d < NES - 1:
                        nc.tensor.matmul(psd_t, lhsT=ones_sb, rhs=es2_tiles[n_d],
                                         start=(n_d == 0), stop=False,
                                         perf_mode=DR)
                        n_d += 1
                    rd_t = rd_pool.tile([P, NW], F32, tag="rd")
                    osb3 = osb_pool.tile([P, CCH, NW], FP8, tag="osb", name="osb")
                    for cs in range(CCH):
                        for i, (p_et, pt) in enumerate(pend):
                            nc.tensor.matmul(
                                pso_tiles[cs],
                                lhsT=vt_sb[pt][:, :, cs * P:(cs + 1) * P],
                                rhs=p_et, start=False, stop=(i == len(pend) - 1),
                                perf_mode=DR)
                        if cs == 0:
                            nc.tensor.matmul(psd_t, lhsT=ones_sb,
                                             rhs=es2_tiles[NES - 1],
                                             start=False, stop=True, perf_mode=DR)
                            nc.vector.reciprocal_approx_fast(out=rd_t, in_=psd_t)
                        # fold the softmax 1/d into the drain
                        nc.vector.tensor_mul(out=osb3[:, cs, :],
                                             in0=pso_tiles[cs], in1=rd_t)
                    state = (osb3, res_t, nch, hs["pbe"])
                    if hook is not None:
                        hook(nch, psx_pool)
            return state

        # ---------------- emit the whole program ----------------
        hs = head_state(0)
        for piece in range(5):
            emit_head_piece(hs, piece, psx_pool)
        carry = None
        for rep in range(reps):
            nxt = head_state(rep + 1) if rep + 1 < reps else None

            def hook(nch, ppool, _n=nxt):
                if _n is not None and nch < 5:
                    emit_head_piece(_n, nch, ppool)

            u8_sb, vt_sb, amt_sb = emit_stage2(hs)
            carry = emit_stage3(hs, u8_sb, vt_sb, amt_sb,
                                hook if nxt is not None else None, carry)
            last_hs = hs
            hs = nxt
        # final projs of the last rep's nch7
        outsb_t = outsb_pool.tile([P, CCH, NW], F32, tag="outsb", name="outsb")
        for oc in range(CCH):
            psp = psx_pool.tile([P, NW], F32, tag="psx", name="psp")
            for ks in (0, 2):
                nc.tensor.matmul(
                    psp, lhsT=pwt_sb[:, ks:ks + 2, oc * P:(oc + 1) * P],
                    rhs=carry[0][:, ks:ks + 2, :],
                    start=(ks == 0), stop=(ks == 2), perf_mode=DR)
            nc.vector.scalar_tensor_tensor(
                out=outsb_t[:, oc, :], in0=psp, scalar=carry[3][:, oc:oc + 1],
                in1=carry[1][:, oc, :], op0=OP.add, op1=OP.add)
        nc.sync.dma_start(
            out=out_d[:, carry[2] * NW:(carry[2] + 1) * NW]
            .rearrange('(c p) w -> p c w', p=P),
            in_=outsb_t)

        for pool in (psx_pool, wsc_pool, stat_pool, res_pool, outsb_pool, rd_pool,
                     osb_pool, es_pool, et_pool, vt_pool, u8_pool, r8_pool,
                     xs_pool, const):
            pool.release()

    nc.compile()
    return nc


def _prep_inputs(x, gn_gamma, gn_beta, q_w, q_b, k_w, k_b, proj_w, proj_b):
    use_amt = bool(np.any(q_b != 0))

    f8 = ml_dtypes.float8_e4m3
    gamma64 = gn_gamma.astype(np.float64)
    # free-side gamma is host-folded into Ws^T; device supplies the rsqrt part
    ws_t = np.ascontiguousarray(
        ((k_w.T.astype(np.float64) @ q_w.astype(np.float64)) * gamma64[None, :])
        .astype(np.float32).astype(f8))
    kwt = np.ascontiguousarray(k_w.T.astype(f8))
    pwt = np.ascontiguousarray(proj_w.T.astype(f8))
    pwk = proj_w.astype(np.float64) @ k_w.astype(np.float64)
    pwkt = np.ascontiguousarray(pwk.T.astype(np.float32).astype(f8))
    # k_b is constant over m and softmax rows sum to 1 -> exact host fold
    pb_host = (proj_b.astype(np.float64)
               + proj_w.astype(np.float64) @ k_b.astype(np.float64)).astype(np.float32)
    gamma_r = np.ascontiguousarray(gn_gamma.reshape(CCH, P).T.astype(np.float32))
    beta64_r = np.ascontiguousarray((gn_beta * 64.0).reshape(CCH, P).T.astype(np.float32))
    pb_r = np.ascontiguousarray(pb_host.reshape(CCH, P).T)
    ones_b = np.ones((P, 2 * P), dtype=f8)
    # g32[p, cp*32+g] = 1 iff channel cp*128+p belongs to group g
    g32 = np.zeros((P, CCH * GROUPS), dtype=np.float32)
    g232 = np.zeros((GROUPS, CCH * P), dtype=np.float32)
    for cp in range(CCH):
        ch = cp * P + np.arange(P)
        g32[np.arange(P), cp * GROUPS + ch // GSIZE] = 1.0
        g232[ch // GSIZE, cp * P + np.arange(P)] = 1.0
    onehot = np.zeros((GROUPS, NW), dtype=np.float32)
    onehot[np.arange(NW) // GSIZE, np.arange(NW)] = 1.0
    ones32 = np.ones((GROUPS, P), dtype=np.float32)

    common = {
        "wst": ws_t, "kwt": kwt, "pwt": pwt, "pwkt": pwkt,
        "gamma_r": gamma_r, "beta64_r": beta64_r, "pb_r": pb_r,
        "ones_b": ones_b, "g32": g32, "g232": g232,
        "onehot": onehot, "ones32": ones32,
    }
    if use_amt:
        common["qbb"] = np.ascontiguousarray(
            np.broadcast_to(q_b.astype(np.float32), (P, NW)))

    in_maps = []
    for i in range(B):
        m = dict(common)
        m["x_in"] = np.ascontiguousarray(x[i].reshape(C, N).astype(np.float32))
        in_maps.append(m)
    return in_maps, use_amt


def kernel(x, gn_gamma, gn_beta, q_w, q_b, k_w, k_b, proj_w, proj_b, _trace=False):
    x = np.asarray(x)
    in_maps, use_amt = _prep_inputs(
        x, np.asarray(gn_gamma), np.asarray(gn_beta), np.asarray(q_w),
        np.asarray(q_b), np.asarray(k_w), np.asarray(k_b),
        np.asarray(proj_w), np.asarray(proj_b))

    key = (use_amt,)
    if key not in _BUILD_CACHE:
        _BUILD_CACHE[key] = _build(use_amt)
    nc = _BUILD_CACHE[key]

    res = bass_utils.run_bass_kernel_spmd(
        nc, in_maps, core_ids=list(range(B)), trace=_trace)
    out = np.stack([r["out"].reshape(C, 64, 64) for r in res.results])
    kernel.last_result = res
    return out.astype(x.dtype)


def make_runner(inputs, chain=1):
    """Build the jitted 8-core executable once; return a callable that runs it
    once and returns wall ns, plus a decoder for the outputs."""
    import time
    import jax
    from jax.experimental.shard_map import shard_map
    from jax.sharding import Mesh, PartitionSpec
    from concourse import bass2jax
    import concourse.mybir as mb

    in_maps, use_amt = _prep_inputs(
        np.asarray(inputs["x"]), np.asarray(inputs["gn_gamma"]),
        np.asarray(inputs["gn_beta"]), np.asarray(inputs["q_w"]),
        np.asarray(inputs["q_b"]), np.asarray(inputs["k_w"]),
        np.asarray(inputs["k_b"]), np.asarray(inputs["proj_w"]),
        np.asarray(inputs["proj_b"]))
    key = (use_amt, chain)
    if key not in _BUILD_CACHE:
        _BUILD_CACHE[key] = _build(use_amt, reps=chain)
    nc = _BUILD_CACHE[key]

    bass2jax.install_neuronx_cc_hook()
    partition_name = nc.partition_id_tensor.name if nc.partition_id_tensor else None
    in_names, out_names, out_avals, zero_outs = [], [], [], []
    for alloc in nc.m.functions[0].allocations:
        if not isinstance(alloc, mb.MemoryLocationSet):
            continue
        name = alloc.memorylocations[0].name
        if alloc.kind == "ExternalInput":
            if name != partition_name:
                in_names.append(name)
        elif alloc.kind == "ExternalOutput":
            out_names.append(name)
            shape = tuple(alloc.tensor_shape)
            dtype = mb.dt.np(alloc.dtype)
            out_avals.append(jax.core.ShapedArray(shape, dtype))
            zero_outs.append(np.zeros(shape, dtype))
    n_params = len(in_names)
    n_outs = len(out_avals)
    all_names = in_names + out_names
    if partition_name is not None:
        all_names = all_names + [partition_name]

    def _body(*args):
        operands = list(args)
        if partition_name is not None:
            operands.append(bass2jax.partition_id_tensor())
        outs = bass2jax._bass_exec_p.bind(
            *operands,
            out_avals=tuple(out_avals),
            in_names=tuple(all_names),
            out_names=tuple(out_names),
            lowering_input_output_aliases=(),
            sim_require_finite=True,
            sim_require_nnan=True,
            nc=nc,
        )
        return tuple(outs)

    donate = tuple(range(n_params, n_params + n_outs))
    devices = jax.devices()[:B]
    mesh = Mesh(np.asarray(devices), ("core",))
    sharded = jax.jit(
        shard_map(_body, mesh=mesh,
                  in_specs=(PartitionSpec("core"),) * (n_params + n_outs),
                  out_specs=(PartitionSpec("core"),) * n_outs,
                  check_rep=False),
        donate_argnums=donate, keep_unused=True)

    concat_in = [
        np.concatenate([np.asarray(in_maps[c][nm]) for c in range(B)], axis=0)
        for nm in in_names
    ]
    concat_zeros = [
        np.zeros((B * z.shape[0], *z.shape[1:]), z.dtype) for z in zero_outs
    ]
    sharding = jax.sharding.NamedSharding(mesh, PartitionSpec("core"))
    dev_in = [jax.device_put(a, sharding) for a in concat_in]

    state = {}

    def run_once():
        dev_zeros = [jax.device_put(z, sharding) for z in concat_zeros]
        for z in dev_zeros:
            z.block_until_ready()
        t0 = time.perf_counter()
        out_arrs = sharded(*dev_in, *dev_zeros)
        for o in out_arrs:
            o.block_until_ready()
        dt = (time.perf_counter() - t0) * 1e9
        state["out_arrs"] = out_arrs
        return dt

    def decode():
        out_arrs = state["out_arrs"]
        return [
            {nm: np.asarray(out_arrs[i]).reshape(B, *out_avals[i].shape)[c]
             for i, nm in enumerate(out_names)}
            for c in range(B)
        ]

    return run_once, decode


def bench(inputs, iters=6, chain=1):
    run_once, decode = make_runner(inputs, chain=chain)
    times = [run_once() for _ in range(iters)]
    return min(times), times, decode()
